# revision 45
# baseline (speedup 1.0000x reference)
"""Sparse multi-head self-attention on 8 trn2 NeuronCores.

Problem: B=4, S=2048, D=768, H=12 heads of 64; only the 512 keys selected by
`uniform_set` (and not padding-masked) participate in attention.

Sharding: core = 2*b + hg  (b = batch 0..3, hg = head-group 0..1, 6 heads each,
Megatron-style column-sharded Wq/Wk/Wv + row-sharded Wo).  Each core computes a
partial output [S, D] for its batch from its 6 heads; host sums the two
head-group partials per batch.

Device algorithm (per core), all layouts transposed so no on-chip transposes:
  Qt[dout, s]  = Wq8^T . X8          fp8 DoubleRow (2 contraction chunks/pass,
                                     3 passes instead of 6 bf16 passes; host
                                     pre-scales both operands by powers of 2,
                                     undone in the PSUM->SBUF copy scale)
  Kt[dout, k]  = WkT . KselT         bf16, chunk-outer so the PE starts on the
                                     first arriving DMA chunk
  V  [k, dout] = VselT^T . WvT       bf16; V block layout per head is
                                     [den | pad63 | v64]: col 0 is the ones
                                     column (softmax denominator -> PSUM row 0,
                                     directly readable by the DVE reciprocal,
                                     which only supports PSUM base partition 0),
                                     v occupies cols 64..127 (64-aligned reads)
  scoresT[k, s] per head (64-contraction, PE row-tiled pairs); expT on ACT
  ctx'T[128, s] = vblk^T . expT   (row 0 = denominator d)
  rp = 1/d (DVE approx recip, reads PSUM row 0), partition-broadcast (gpsimd),
  ctxT = ctx'T[64:128] * rbb  (DVE, bf16 out)
  out partial[s_chunk, dout] = ctxT^T . WoT

Host pre-permutes every operand to the device SBUF layout so all input DMAs
are fully contiguous, and splits K-path DMAs into chunk pairs so the first
matmul can start ~1us after launch.

Biases: bq assumed 0 (reference generates zeros).  bk affects scores only via
per-query constants (softmax invariant).  bv and bo are applied exactly on the
host: out += bo + Wo @ bv (softmax weights sum to 1).
"""

import numpy as np

B, S, D, H, HD = 4, 2048, 768, 12, 64
HG = 2            # head groups (tensor parallel)
HPG = H // HG     # 6 heads per group
DG = HPG * HD     # 384 projection dims per group
NK = 512          # padded count of selected keys
P = 128
KC = D // P       # 6 contraction chunks over model dim
MC = DG // P      # 3 chunks of per-group projection dim
SC = NK // P      # 4 selected-key chunks
SQT = 512         # query-tile (moving free dim)
NSQT = S // SQT   # 4
NPAIR = MC        # 3 head-pairs per tile (pair p = heads 2p, 2p+1 in chunk p)

# fp8 Q-projection scales (powers of 2; product undone at PSUM evacuation)
SX8 = 8.0         # xt8 = x * SX8          (x std 1 -> 8)
SW8 = 4096.0      # wqt8 = Wq.T*scale*SW8  (std 0.0025 -> 10)
QEVAC = 1.0 / (SX8 * SW8)

_CACHE = {}


def _build_bass(with_bias):
    import concourse.mybir as mybir
    import concourse.tile as tile
    from concourse import bacc

    f32 = mybir.dt.float32
    bf16 = mybir.dt.bfloat16
    fp8 = mybir.dt.float8e4
    EXP = mybir.ActivationFunctionType.Exp
    COPY = mybir.ActivationFunctionType.Copy
    DRMODE = mybir.MatmulPerfMode.DoubleRow

    nc = bacc.Bacc("TRN2", name="sparse_mha")

    xt8_d = nc.dram_tensor("xt8", [NSQT, P, KC, SQT], fp8, kind="ExternalInput")
    wqt8_d = nc.dram_tensor("wqt8", [P, KC, DG], fp8, kind="ExternalInput")
    wkt_d = nc.dram_tensor("wkt", [P, KC, DG], bf16, kind="ExternalInput")
    kselt_d = nc.dram_tensor("kselt", [2, P, KC, NK // 2], bf16, kind="ExternalInput")
    wvt_d = nc.dram_tensor("wvt", [P, KC, DG], bf16, kind="ExternalInput")
    vselt_d = nc.dram_tensor("vselt", [P, KC, NK], bf16, kind="ExternalInput")
    wot_d = nc.dram_tensor("wot", [P, MC, D], bf16, kind="ExternalInput")
    kb_d = nc.dram_tensor("kbias", [P, SC], f32, kind="ExternalInput") if with_bias else None
    out_d = nc.dram_tensor("out", [S, D], bf16, kind="ExternalOutput")

    with tile.TileContext(nc) as tc:
        with (
            tc.tile_pool(name="persist", bufs=1) as persist,
            tc.tile_pool(name="inputs", bufs=1) as inputs,
            tc.tile_pool(name="work", bufs=1) as work,
            tc.tile_pool(name="psum", bufs=1, space="PSUM") as psum,
        ):
            # ---- input DMAs: K path first (chunk pairs across two queues) so
            # the first matmuls start early; Q path on gpsimd queue in
            # parallel; V/Wo behind them ----
            if with_bias:
                kbias = persist.tile([P, SC], f32, tag="kbias")
                nc.gpsimd.dma_start(kbias, kb_d[:, :])

            # Critical path to wave-0 scores: kselt (full, chunk-paced) +
            # wkt m=0 slab + xt8 tile-0 + wqt8.  The m=1,2 K/Q work and V
            # are consumed later, inside wave 0.
            wkt = inputs.tile([P, KC, DG], bf16, tag="wkt")
            kselt = inputs.tile([P, 2, KC, NK // 2], bf16, tag="kselt")
            wqt8 = inputs.tile([P, KC, DG], fp8, tag="wqt8")
            xt8 = inputs.tile([P, NSQT, KC, SQT], fp8, tag="xt8")
            # key-half 0 of kselt first: wave-0 scores j=0 only needs keys
            # 0:256, so K-proj(half 0) + Q(t0) gate the pipeline start
            nc.sync.dma_start(kselt[:, 0, 0:2, :], kselt_d[0, :, 0:2, :])
            nc.scalar.dma_start(wkt[:, 0:2, :], wkt_d[:, 0:2, :])
            nc.gpsimd.dma_start(wkt[:, 2:4, :], wkt_d[:, 2:4, :])
            nc.sync.dma_start(kselt[:, 0, 2:4, :], kselt_d[0, :, 2:4, :])
            nc.scalar.dma_start(wkt[:, 4:6, :], wkt_d[:, 4:6, :])
            nc.gpsimd.dma_start(kselt[:, 0, 4:6, :], kselt_d[0, :, 4:6, :])

            nc.scalar.dma_start(xt8[:, 0, 3:6, :], xt8_d[0, :, 3:6, :])
            nc.gpsimd.dma_start(wqt8, wqt8_d[:, :, :])
            nc.sync.dma_start(xt8[:, 0, 0:3, :], xt8_d[0, :, 0:3, :])

            nc.sync.dma_start(kselt[:, 1, 0:3, :], kselt_d[1, :, 0:3, :])
            nc.gpsimd.dma_start(kselt[:, 1, 3:6, :], kselt_d[1, :, 3:6, :])

            # V path, later query tiles
            wvt = inputs.tile([P, KC, DG], bf16, tag="wvt")
            vselt = inputs.tile([P, KC, NK], bf16, tag="vselt")
            nc.sync.dma_start(vselt[:, 0:3, :], vselt_d[:, 0:3, :])
            nc.gpsimd.dma_start(vselt[:, 3:6, :], vselt_d[:, 3:6, :])
            nc.scalar.dma_start(wvt, wvt_d[:, :, :])

            wot = persist.tile([P, MC, D], bf16, tag="wot")
            nc.gpsimd.dma_start(xt8[:, 1, :, :], xt8_d[1, :, :, :])
            nc.scalar.dma_start(wot, wot_d[:, :, :])
            nc.sync.dma_start(xt8[:, 2, :, :], xt8_d[2, :, :, :])
            nc.scalar.dma_start(xt8[:, 3, :, :], xt8_d[3, :, :, :])

            # V blocks: [P(sk), SC, HPG, 128]; col 0 = ones (denominator row),
            # cols 1..63 zero pad, cols 64..128 = v
            vb = persist.tile([P, SC, HPG, P], bf16, tag="vb")
            nc.vector.memset(vb[:, :, :, 0:HD], 0.0)
            nc.vector.memset(vb[:, :, :, 0:1], 1.0)

            ktp = persist.tile([P, MC, NK], bf16, tag="ktp")
            qt = persist.tile([P, MC, S], bf16, tag="qt")

            # ---- warmup projections ----
            # K: chunk-outer over 3 open PSUM groups, paced by the kselt
            # chunk DMAs
            psK = [psum.tile([P, SQT], f32, tag="ctx", bufs=3, name=f"kp{m}") for m in range(MC)]
            HK = NK // 2
            for h in range(2):
                for i in range(KC):
                    for m in range(MC):
                        nc.tensor.matmul(
                            psK[m][:, h * HK : (h + 1) * HK],
                            lhsT=wkt[:, i, m * P : (m + 1) * P],
                            rhs=kselt[:, h, i, :],
                            start=(i == 0),
                            stop=(i == KC - 1),
                        )
                for m in range(MC):
                    nc.vector.tensor_copy(
                        ktp[:, m, h * HK : (h + 1) * HK],
                        psK[m][:, h * HK : (h + 1) * HK],
                    )

            def qproj_group(m, t):
                sq = slice(t * SQT, (t + 1) * SQT)
                ps = psum.tile([P, SQT], f32, tag="pj", bufs=1, name=f"qp{t}_{m}")
                for j in range(MC):
                    nc.tensor.matmul(
                        ps[:, :SQT],
                        lhsT=wqt8[:, 2 * j : 2 * j + 2, m * P : (m + 1) * P],
                        rhs=xt8[:, t, 2 * j : 2 * j + 2, :],
                        start=(j == 0),
                        stop=(j == MC - 1),
                        perf_mode=DRMODE,
                    )
                nc.scalar.activation(qt[:, m, sq], ps[:, :SQT], COPY, 0.0, QEVAC)

            for m in range(MC):
                qproj_group(m, 0)

            def vproj():
                # V: chunk-outer, 4 open groups in the 2 sc-ring tiles
                psV = [psum.tile([P, 2 * SQT], f32, tag="sc", bufs=2, name=f"vp{cc}") for cc in range(2)]
                for i in range(KC):
                    for c in range(SC):
                        nc.tensor.matmul(
                            psV[c // 2][:, (c % 2) * SQT : (c % 2) * SQT + DG],
                            lhsT=vselt[:, i, c * P : (c + 1) * P],
                            rhs=wvt[:, i, :],
                            start=(i == 0),
                            stop=(i == KC - 1),
                        )
                for c in range(SC):
                    nc.vector.tensor_copy(
                        vb[:, c, :, HD:P],
                        psV[c // 2][:, (c % 2) * SQT : (c % 2) * SQT + DG].rearrange(
                            "p (h d) -> p h d", h=HPG
                        ),
                    )

            # ---- out-projection (per query-chunk of 128); output DMAs
            # alternate between the sync and gpsimd rings so the final
            # cluster drains in parallel ----
            op_count = [0]

            def outproj_group(t_o, ctxt_o, mq, evac=None):
                sq0 = t_o * SQT + mq * P
                op = psum.tile([P, 2 * SQT], f32, tag="sc", bufs=2, name=f"op{t_o}_{mq}")
                for n in range(2):
                    for j2 in range(MC):
                        nc.tensor.matmul(
                            op[:, n * SQT : n * SQT + 384],
                            lhsT=ctxt_o[:, j2, mq * P : (mq + 1) * P],
                            rhs=wot[:, j2, n * 384 : (n + 1) * 384],
                            start=(j2 == 0),
                            stop=(j2 == MC - 1),
                        )
                ot = work.tile([P, D], bf16, tag="ot", bufs=4, name=f"ot{t_o}_{mq}")
                src = op.rearrange("p (n x) -> p n x", n=2)[:, :, 0:384]
                dst = ot.rearrange("p (n x) -> p n x", n=2)
                if evac is None:
                    nc.vector.tensor_copy(dst, src)
                else:
                    evac.copy(dst, src)
                op_count[0] += 1
                if evac is not None:
                    # drain phase: split across both rings to halve the tail
                    nc.sync.dma_start(out_d[sq0 : sq0 + 64, :], ot[0:64, :])
                    nc.gpsimd.dma_start(out_d[sq0 + 64 : sq0 + P, :], ot[64:P, :])
                else:
                    eng = nc.sync if op_count[0] % 2 == 0 else nc.gpsimd
                    eng.dma_start(out_d[sq0 : sq0 + P, :], ot)

            # ---- attention waves ----
            NW = NSQT * NPAIR

            def scores_cpair(w, j, ets_pair):
                t, p = divmod(w, NPAIR)
                sq = slice(t * SQT, (t + 1) * SQT)
                sct = [
                    psum.tile([P, 2 * SQT], f32, tag="sc", bufs=2, name=f"sc{w}_{j}_{hi}")
                    for hi in range(2)
                ]
                for cc in range(2):
                    c = 2 * j + cc
                    for hi in range(2):
                        lo = 64 * hi
                        nc.tensor.matmul(
                            sct[hi][:, cc * SQT : (cc + 1) * SQT],
                            lhsT=ktp[lo : lo + 64, p, c * P : (c + 1) * P],
                            rhs=qt[lo : lo + 64, p, sq],
                            start=True,
                            stop=True,
                        )
                for hi in range(2):
                    if with_bias:
                        for cc in range(2):
                            c = 2 * j + cc
                            nc.scalar.activation(
                                out=ets_pair[hi][:, c, :],
                                in_=sct[hi][:, cc * SQT : (cc + 1) * SQT],
                                func=EXP,
                                bias=kbias[:, c : c + 1],
                                scale=1.0,
                            )
                    else:
                        nc.scalar.activation(
                            out=ets_pair[hi][:, 2 * j : 2 * j + 2, :],
                            in_=sct[hi].rearrange("p (c q) -> p c q", c=2),
                            func=EXP,
                        )

            def ctx_part(w, hi, ets_pair, state, cs):
                t, p = divmod(w, NPAIR)
                h = 2 * p + hi
                if cs[0] == 0:
                    state["cp"][hi] = psum.tile(
                        [P, SQT], f32, tag="ctx", bufs=3, name=f"cp{w}_{hi}"
                    )
                cp = state["cp"][hi]
                for c in cs:
                    nc.tensor.matmul(
                        cp[:, :],
                        lhsT=vb[:, c, h, :],
                        rhs=ets_pair[hi][:, c, :],
                        start=(c == 0),
                        stop=(c == SC - 1),
                    )
                if cs[-1] != SC - 1:
                    return
                # 1/d directly from PSUM row 0 (DVE), partition-broadcast
                sl = slice(hi * SQT, (hi + 1) * SQT)
                rp, rbb = state["rp"], state["rbb"]
                nc.vector.reciprocal_approx_fast(rp[0:1, sl], cp[0:1, :])
                nc.gpsimd.partition_broadcast(rbb[0:HD, sl], rp[0:1, sl])

            def ctx_head(w, hi, ets_pair, state):
                ctx_part(w, hi, ets_pair, state, [0, 1])
                ctx_part(w, hi, ets_pair, state, [2, 3])

            def norm_head(w, hi, state):
                t, p = divmod(w, NPAIR)
                lo = 64 * hi
                nc.vector.tensor_tensor(
                    state["ctxt"][lo : lo + 64, p, :],
                    state["cp"][hi][64:P, :],
                    state["rbb"][0:HD, hi * SQT : (hi + 1) * SQT],
                    op=mybir.AluOpType.mult,
                )

            # steady-state emission: per window issue scores(w), retire w-1,
            # weave outproj of tile t-1 and one qproj group for tile t+1
            op_queue = []
            prev = None
            for w in range(NW):
                t, p = divmod(w, NPAIR)
                ets_pair = [
                    work.tile([P, SC, SQT], bf16, tag="ets", bufs=6, name=f"et{w}_{hi}")
                    for hi in range(2)
                ]
                state = {
                    "rp": work.tile([1, 2 * SQT], f32, tag="rp", bufs=3, name=f"rp{w}"),
                    "rbb": work.tile([HD, 2 * SQT], f32, tag="rbb", bufs=3, name=f"rbb{w}"),
                    "cp": [None, None],
                    "ctxt": (
                        prev["ctxt"]
                        if p != 0
                        else work.tile([P, MC, SQT], bf16, tag="ctxt", bufs=4, name=f"ctxt{t}")
                    ),
                }

                scores_cpair(w, 0, ets_pair)
                if prev is not None:
                    ctx_head(w - 1, 0, prev["ets"], prev)
                scores_cpair(w, 1, ets_pair)
                if w == 0:
                    # V projection emitted behind wave-0 scores: its DMAs
                    # arrive while K/Q/scores keep the PE busy and its
                    # matmuls fill the PE while wave-0 exp streams
                    vproj()
                if prev is not None:
                    norm_head(w - 1, 0, prev)
                    ctx_head(w - 1, 1, prev["ets"], prev)
                    norm_head(w - 1, 1, prev)
                    if (w - 1) % NPAIR == NPAIR - 1:
                        for mq in range(SQT // P):
                            op_queue.append(((w - 1) // NPAIR, prev["ctxt"], mq))
                pops = 0 if t == NSQT - 1 else 2
                for _ in range(pops):
                    if op_queue:
                        t_o, ctxt_o, mq = op_queue.pop(0)
                        outproj_group(t_o, ctxt_o, mq)
                if t + 1 < NSQT:
                    qproj_group(p, t + 1)

                prev = {"ets": ets_pair, **state}

            # drain: retire the last wave, interleaving the held-back tile-2
            # outproj groups between the normalization chain stages so the PE
            # stays busy while the last chains run
            w = NW - 1

            def pop_ops(k):
                for _ in range(k):
                    if op_queue:
                        t_o, ctxt_o, mq = op_queue.pop(0)
                        outproj_group(t_o, ctxt_o, mq, evac=nc.scalar)

            ctx_head(w, 0, prev["ets"], prev)
            pop_ops(1)
            ctx_head(w, 1, prev["ets"], prev)
            norm_head(w, 0, prev)
            pop_ops(2)
            norm_head(w, 1, prev)
            pop_ops(len(op_queue))
            for mq in range(SQT // P):
                outproj_group(NSQT - 1, prev["ctxt"], mq, evac=nc.scalar)

    nc.compile()
    return nc


def _get_nc(with_bias):
    key = ("bias" if with_bias else "fast")
    if key not in _CACHE:
        _CACHE[key] = _build_bass(with_bias)
    return _CACHE[key]


def _chunked(a):
    """[D, N] -> [P, D//P, N] with d = o*P + p  ->  [p][o][n], contiguous."""
    Dd, N = a.shape
    return np.ascontiguousarray(a.reshape(Dd // P, P, N).transpose(1, 0, 2))


def kernel(query, key, value, mask, uniform_set, Wq, bq, Wk, bk, Wv, bv, Wo, bo):
    import ml_dtypes
    from concourse import bass_utils

    bft = ml_dtypes.bfloat16
    f8t = ml_dtypes.float8_e4m3

    query = np.asarray(query, dtype=np.float32)
    key = np.asarray(key, dtype=np.float32)
    value = np.asarray(value, dtype=np.float32)
    mask = np.asarray(mask, dtype=np.float32)
    us = np.asarray(uniform_set).astype(bool)
    Wq = np.asarray(Wq, dtype=np.float32)
    Wk = np.asarray(Wk, dtype=np.float32)
    Wv = np.asarray(Wv, dtype=np.float32)
    Wo = np.asarray(Wo, dtype=np.float32)
    bq = np.asarray(bq, dtype=np.float32)
    bk = np.asarray(bk, dtype=np.float32)
    bv = np.asarray(bv, dtype=np.float32)
    bo = np.asarray(bo, dtype=np.float32)
    assert np.all(bq == 0.0), "kernel assumes bq == 0 (reference generates zeros)"

    scale = 1.0 / float(HD) ** 0.5
    wqt8_g = [
        _chunked(np.clip(Wq.T[:, g * DG : (g + 1) * DG] * (scale * SW8), -240, 240)).astype(f8t)
        for g in range(HG)
    ]
    wkt_g = [_chunked(Wk.T[:, g * DG : (g + 1) * DG]).astype(bft) for g in range(HG)]
    wvt_g = [_chunked(Wv.T[:, g * DG : (g + 1) * DG]).astype(bft) for g in range(HG)]
    wot_g = [
        np.ascontiguousarray(
            Wo.T[g * DG : (g + 1) * DG, :].reshape(MC, P, D).transpose(1, 0, 2)
        ).astype(bft)
        for g in range(HG)
    ]

    keeps = [us & (mask[b, 0, 0] >= 0) for b in range(B)]
    ns = [int(k.sum()) for k in keeps]
    with_bias = any(n < NK for n in ns)
    assert all(0 < n <= NK for n in ns), f"selected key counts {ns} unsupported"
    nc = _get_nc(with_bias)

    in_maps = []
    for b in range(B):
        idx = np.nonzero(keeps[b])[0]
        n = len(idx)
        kselt = np.zeros((D, NK), np.float32)
        kselt[:, :n] = key[b][idx].T
        vselt = np.zeros((D, NK), np.float32)
        vselt[:, :n] = value[b][idx].T
        kselt = np.ascontiguousarray(
            _chunked(kselt).reshape(P, KC, 2, NK // 2).transpose(2, 0, 1, 3)
        ).astype(bft)
        vselt = _chunked(vselt).astype(bft)
        # xt8: [NSQT, P, KC, SQT] fp8 (x * SX8)
        xt = _chunked(np.clip(query[b].T * SX8, -240, 240))  # [P, KC, S]
        xt8 = np.ascontiguousarray(
            xt.reshape(P, KC, NSQT, SQT).transpose(2, 0, 1, 3)
        ).astype(f8t)
        for g in range(HG):
            m = {
                "xt8": xt8,
                "kselt": kselt,
                "vselt": vselt,
                "wqt8": wqt8_g[g],
                "wkt": wkt_g[g],
                "wvt": wvt_g[g],
                "wot": wot_g[g],
            }
            if with_bias:
                kbias = np.full((NK,), -1e30, np.float32)
                kbias[:n] = 0.0
                m["kbias"] = np.ascontiguousarray(kbias.reshape(SC, P).T)
            in_maps.append(m)

    res = bass_utils.run_bass_kernel_spmd(nc, in_maps, core_ids=list(range(B * HG)))
    outs = [m["out"] for m in res.results]

    corr = (bo + Wo @ bv).astype(np.float32)
    out = np.empty((B, S, D), np.float32)
    for b in range(B):
        out[b] = (
            outs[HG * b].astype(np.float32)
            + outs[HG * b + 1].astype(np.float32)
            + corr
        )
    return out


# revision 46
# speedup vs baseline: 1.1870x; 1.1870x over previous
"""Sparse multi-head self-attention on 8 trn2 NeuronCores.

Problem: B=4, S=2048, D=768, H=12 heads of 64; only the 512 keys selected by
`uniform_set` (and not padding-masked) participate in attention.

Sharding: core = 2*b + hg  (b = batch 0..3, hg = head-group 0..1, 6 heads each,
Megatron-style column-sharded Wq/Wk/Wv + row-sharded Wo).  Each core computes a
partial output [S, D] for its batch from its 6 heads; host sums the two
head-group partials per batch.

Device algorithm (per core), all layouts transposed so no on-chip transposes:
  Qt[dout, s]  = Wq8^T . X8          fp8 DoubleRow (2 contraction chunks/pass,
                                     3 passes instead of 6 bf16 passes; host
                                     pre-scales both operands by powers of 2,
                                     undone in the PSUM->SBUF copy scale)
  Kt[dout, k]  = WkT . KselT         bf16, chunk-outer so the PE starts on the
                                     first arriving DMA chunk
  V  [k, dout] = VselT^T . WvT       bf16; V block layout per head is
                                     [den | pad63 | v64]: col 0 is the ones
                                     column (softmax denominator -> PSUM row 0,
                                     directly readable by the DVE reciprocal,
                                     which only supports PSUM base partition 0),
                                     v occupies cols 64..127 (64-aligned reads)
  scoresT[k, s] per head (64-contraction, PE row-tiled pairs); expT on ACT
  ctx'T[128, s] = vblk^T . expT   (row 0 = denominator d)
  rp = 1/d (DVE approx recip, reads PSUM row 0), partition-broadcast (gpsimd),
  ctxT = ctx'T[64:128] * rbb  (DVE, bf16 out)
  out partial[s_chunk, dout] = ctxT^T . WoT

Host pre-permutes every operand to the device SBUF layout so all input DMAs
are fully contiguous, and splits K-path DMAs into chunk pairs so the first
matmul can start ~1us after launch.

Biases: bq assumed 0 (reference generates zeros).  bk affects scores only via
per-query constants (softmax invariant).  bv and bo are applied exactly on the
host: out += bo + Wo @ bv (softmax weights sum to 1).
"""

import numpy as np

B, S, D, H, HD = 4, 2048, 768, 12, 64
HG = 2            # head groups (tensor parallel)
HPG = H // HG     # 6 heads per group
DG = HPG * HD     # 384 projection dims per group
NK = 512          # padded count of selected keys
P = 128
KC = D // P       # 6 contraction chunks over model dim
MC = DG // P      # 3 chunks of per-group projection dim
SC = NK // P      # 4 selected-key chunks
SQT = 512         # query-tile (moving free dim)
NSQT = S // SQT   # 4
NPAIR = MC        # 3 head-pairs per tile (pair p = heads 2p, 2p+1 in chunk p)

# fp8 Q-projection scales (powers of 2; product undone at PSUM evacuation)
SX8 = 8.0         # xt8 = x * SX8          (x std 1 -> 8)
SW8 = 4096.0      # wqt8 = Wq.T*scale*SW8  (std 0.0025 -> 10)
QEVAC = 1.0 / (SX8 * SW8)

_CACHE = {}


def _build_bass(with_bias):
    import concourse.mybir as mybir
    import concourse.tile as tile
    from concourse import bacc

    f32 = mybir.dt.float32
    bf16 = mybir.dt.bfloat16
    fp8 = mybir.dt.float8e4
    EXP = mybir.ActivationFunctionType.Exp
    COPY = mybir.ActivationFunctionType.Copy
    DRMODE = mybir.MatmulPerfMode.DoubleRow

    nc = bacc.Bacc("TRN2", name="sparse_mha")

    xt8_d = nc.dram_tensor("xt8", [NSQT, P, KC, SQT], fp8, kind="ExternalInput")
    wqt8_d = nc.dram_tensor("wqt8", [P, KC, DG], fp8, kind="ExternalInput")
    wkt_d = nc.dram_tensor("wkt", [P, KC, DG], bf16, kind="ExternalInput")
    kselt_d = nc.dram_tensor("kselt", [P, KC, NK], bf16, kind="ExternalInput")
    wvt_d = nc.dram_tensor("wvt", [P, KC, DG], bf16, kind="ExternalInput")
    vselt_d = nc.dram_tensor("vselt", [P, KC, NK], bf16, kind="ExternalInput")
    wot_d = nc.dram_tensor("wot", [P, MC, D], bf16, kind="ExternalInput")
    kb_d = nc.dram_tensor("kbias", [P, SC], f32, kind="ExternalInput") if with_bias else None
    out_d = nc.dram_tensor("out", [S, D], bf16, kind="ExternalOutput")

    with tile.TileContext(nc) as tc:
        with (
            tc.tile_pool(name="persist", bufs=1) as persist,
            tc.tile_pool(name="inputs", bufs=1) as inputs,
            tc.tile_pool(name="work", bufs=1) as work,
            tc.tile_pool(name="psum", bufs=1, space="PSUM") as psum,
        ):
            # ---- input DMAs: K path first (chunk pairs across two queues) so
            # the first matmuls start early; Q path on gpsimd queue in
            # parallel; V/Wo behind them ----
            if with_bias:
                kbias = persist.tile([P, SC], f32, tag="kbias")
                nc.gpsimd.dma_start(kbias, kb_d[:, :])

            # Critical path to wave-0 scores: kselt (full, chunk-paced) +
            # wkt m=0 slab + xt8 tile-0 + wqt8.  The m=1,2 K/Q work and V
            # are consumed later, inside wave 0.
            wkt = inputs.tile([P, KC, DG], bf16, tag="wkt")
            kselt = inputs.tile([P, KC, NK], bf16, tag="kselt")
            wqt8 = inputs.tile([P, KC, DG], fp8, tag="wqt8")
            xt8 = inputs.tile([P, NSQT, KC, SQT], fp8, tag="xt8")
            nc.sync.dma_start(kselt[:, 0:1, :], kselt_d[:, 0:1, :])
            nc.scalar.dma_start(wkt[:, 0:1, :], wkt_d[:, 0:1, :])
            nc.gpsimd.dma_start(wkt[:, 2:4, :], wkt_d[:, 2:4, :])
            nc.sync.dma_start(kselt[:, 1:2, :], kselt_d[:, 1:2, :])
            nc.scalar.dma_start(wkt[:, 1:2, :], wkt_d[:, 1:2, :])
            nc.sync.dma_start(kselt[:, 2:4, :], kselt_d[:, 2:4, :])
            nc.gpsimd.dma_start(wkt[:, 4:6, :], wkt_d[:, 4:6, :])
            nc.gpsimd.dma_start(kselt[:, 4:6, :], kselt_d[:, 4:6, :])

            nc.scalar.dma_start(xt8[:, 0, 3:6, :], xt8_d[0, :, 3:6, :])
            nc.gpsimd.dma_start(wqt8, wqt8_d[:, :, :])
            nc.sync.dma_start(xt8[:, 0, 0:3, :], xt8_d[0, :, 0:3, :])

            # V path, later query tiles
            wvt = inputs.tile([P, KC, DG], bf16, tag="wvt")
            vselt = inputs.tile([P, KC, NK], bf16, tag="vselt")
            nc.sync.dma_start(vselt[:, 0:3, :], vselt_d[:, 0:3, :])
            nc.gpsimd.dma_start(vselt[:, 3:6, :], vselt_d[:, 3:6, :])
            nc.scalar.dma_start(wvt, wvt_d[:, :, :])

            wot = persist.tile([P, MC, D], bf16, tag="wot")
            nc.gpsimd.dma_start(xt8[:, 1, :, :], xt8_d[1, :, :, :])
            nc.scalar.dma_start(wot, wot_d[:, :, :])
            nc.sync.dma_start(xt8[:, 2, :, :], xt8_d[2, :, :, :])
            nc.scalar.dma_start(xt8[:, 3, :, :], xt8_d[3, :, :, :])

            # V blocks: [P(sk), SC, HPG, 128]; col 0 = ones (denominator row),
            # cols 1..63 zero pad, cols 64..128 = v
            vb = persist.tile([P, SC, HPG, P], bf16, tag="vb")
            nc.vector.memset(vb[:, :, :, 0:HD], 0.0)
            nc.vector.memset(vb[:, :, :, 0:1], 1.0)

            ktp = persist.tile([P, MC, NK], bf16, tag="ktp")
            qt = persist.tile([P, MC, S], bf16, tag="qt")

            # ---- warmup projections ----
            # K: chunk-outer over 3 open PSUM groups, paced by the kselt
            # chunk DMAs
            psK = [psum.tile([P, SQT], f32, tag="ctx", bufs=3, name=f"kp{m}") for m in range(MC)]
            for i in range(KC):
                for m in range(MC):
                    nc.tensor.matmul(
                        psK[m][:, :NK],
                        lhsT=wkt[:, i, m * P : (m + 1) * P],
                        rhs=kselt[:, i, :],
                        start=(i == 0),
                        stop=(i == KC - 1),
                    )
            for m in range(MC):
                nc.vector.tensor_copy(ktp[:, m, :], psK[m][:, :NK])

            def qproj_group(m, t):
                sq = slice(t * SQT, (t + 1) * SQT)
                ps = psum.tile([P, SQT], f32, tag="pj", bufs=1, name=f"qp{t}_{m}")
                for j in range(MC):
                    nc.tensor.matmul(
                        ps[:, :SQT],
                        lhsT=wqt8[:, 2 * j : 2 * j + 2, m * P : (m + 1) * P],
                        rhs=xt8[:, t, 2 * j : 2 * j + 2, :],
                        start=(j == 0),
                        stop=(j == MC - 1),
                        perf_mode=DRMODE,
                    )
                nc.scalar.activation(qt[:, m, sq], ps[:, :SQT], COPY, 0.0, QEVAC)

            for m in range(MC):
                qproj_group(m, 0)

            def vproj():
                # V: chunk-outer, 4 open groups in the 2 sc-ring tiles
                psV = [psum.tile([P, 2 * SQT], f32, tag="sc", bufs=2, name=f"vp{cc}") for cc in range(2)]
                for i in range(KC):
                    for c in range(SC):
                        nc.tensor.matmul(
                            psV[c // 2][:, (c % 2) * SQT : (c % 2) * SQT + DG],
                            lhsT=vselt[:, i, c * P : (c + 1) * P],
                            rhs=wvt[:, i, :],
                            start=(i == 0),
                            stop=(i == KC - 1),
                        )
                for c in range(SC):
                    nc.vector.tensor_copy(
                        vb[:, c, :, HD:P],
                        psV[c // 2][:, (c % 2) * SQT : (c % 2) * SQT + DG].rearrange(
                            "p (h d) -> p h d", h=HPG
                        ),
                    )

            # ---- out-projection (per query-chunk of 128); output DMAs
            # alternate between the sync and gpsimd rings so the final
            # cluster drains in parallel ----
            op_count = [0]

            def outproj_group(t_o, ctxt_o, mq, evac=None):
                sq0 = t_o * SQT + mq * P
                op = psum.tile([P, 2 * SQT], f32, tag="sc", bufs=2, name=f"op{t_o}_{mq}")
                for n in range(2):
                    for j2 in range(MC):
                        nc.tensor.matmul(
                            op[:, n * SQT : n * SQT + 384],
                            lhsT=ctxt_o[:, j2, mq * P : (mq + 1) * P],
                            rhs=wot[:, j2, n * 384 : (n + 1) * 384],
                            start=(j2 == 0),
                            stop=(j2 == MC - 1),
                        )
                ot = work.tile([P, D], bf16, tag="ot", bufs=4, name=f"ot{t_o}_{mq}")
                src = op.rearrange("p (n x) -> p n x", n=2)[:, :, 0:384]
                dst = ot.rearrange("p (n x) -> p n x", n=2)
                if evac is None:
                    nc.vector.tensor_copy(dst, src)
                else:
                    evac.copy(dst, src)
                op_count[0] += 1
                if evac is not None:
                    # drain phase: split across both rings to halve the tail
                    nc.sync.dma_start(out_d[sq0 : sq0 + 64, :], ot[0:64, :])
                    nc.gpsimd.dma_start(out_d[sq0 + 64 : sq0 + P, :], ot[64:P, :])
                else:
                    eng = nc.sync if op_count[0] % 2 == 0 else nc.gpsimd
                    eng.dma_start(out_d[sq0 : sq0 + P, :], ot)

            # ---- attention waves ----
            NW = NSQT * NPAIR

            def scores_cpair(w, j, ets_pair):
                t, p = divmod(w, NPAIR)
                sq = slice(t * SQT, (t + 1) * SQT)
                sct = [
                    psum.tile([P, 2 * SQT], f32, tag="sc", bufs=2, name=f"sc{w}_{j}_{hi}")
                    for hi in range(2)
                ]
                for cc in range(2):
                    c = 2 * j + cc
                    for hi in range(2):
                        lo = 64 * hi
                        nc.tensor.matmul(
                            sct[hi][:, cc * SQT : (cc + 1) * SQT],
                            lhsT=ktp[lo : lo + 64, p, c * P : (c + 1) * P],
                            rhs=qt[lo : lo + 64, p, sq],
                            start=True,
                            stop=True,
                        )
                for hi in range(2):
                    if with_bias:
                        for cc in range(2):
                            c = 2 * j + cc
                            nc.scalar.activation(
                                out=ets_pair[hi][:, c, :],
                                in_=sct[hi][:, cc * SQT : (cc + 1) * SQT],
                                func=EXP,
                                bias=kbias[:, c : c + 1],
                                scale=1.0,
                            )
                    else:
                        nc.scalar.activation(
                            out=ets_pair[hi][:, 2 * j : 2 * j + 2, :],
                            in_=sct[hi].rearrange("p (c q) -> p c q", c=2),
                            func=EXP,
                        )

            def ctx_part(w, hi, ets_pair, state, cs):
                t, p = divmod(w, NPAIR)
                h = 2 * p + hi
                if cs[0] == 0:
                    state["cp"][hi] = psum.tile(
                        [P, SQT], f32, tag="ctx", bufs=3, name=f"cp{w}_{hi}"
                    )
                cp = state["cp"][hi]
                for c in cs:
                    nc.tensor.matmul(
                        cp[:, :],
                        lhsT=vb[:, c, h, :],
                        rhs=ets_pair[hi][:, c, :],
                        start=(c == 0),
                        stop=(c == SC - 1),
                    )
                if cs[-1] != SC - 1:
                    return
                # 1/d directly from PSUM row 0 (DVE), partition-broadcast
                sl = slice(hi * SQT, (hi + 1) * SQT)
                rp, rbb = state["rp"], state["rbb"]
                nc.vector.reciprocal_approx_fast(rp[0:1, sl], cp[0:1, :])
                nc.gpsimd.partition_broadcast(rbb[0:HD, sl], rp[0:1, sl])

            def ctx_head(w, hi, ets_pair, state):
                ctx_part(w, hi, ets_pair, state, [0, 1])
                ctx_part(w, hi, ets_pair, state, [2, 3])

            def norm_head(w, hi, state):
                t, p = divmod(w, NPAIR)
                lo = 64 * hi
                nc.vector.tensor_tensor(
                    state["ctxt"][lo : lo + 64, p, :],
                    state["cp"][hi][64:P, :],
                    state["rbb"][0:HD, hi * SQT : (hi + 1) * SQT],
                    op=mybir.AluOpType.mult,
                )

            # steady-state emission: per window issue scores(w), retire w-1,
            # weave outproj of tile t-1 and one qproj group for tile t+1
            op_queue = []
            prev = None
            for w in range(NW):
                t, p = divmod(w, NPAIR)
                ets_pair = [
                    work.tile([P, SC, SQT], bf16, tag="ets", bufs=6, name=f"et{w}_{hi}")
                    for hi in range(2)
                ]
                state = {
                    "rp": work.tile([1, 2 * SQT], f32, tag="rp", bufs=3, name=f"rp{w}"),
                    "rbb": work.tile([HD, 2 * SQT], f32, tag="rbb", bufs=3, name=f"rbb{w}"),
                    "cp": [None, None],
                    "ctxt": (
                        prev["ctxt"]
                        if p != 0
                        else work.tile([P, MC, SQT], bf16, tag="ctxt", bufs=4, name=f"ctxt{t}")
                    ),
                }

                scores_cpair(w, 0, ets_pair)
                if prev is not None:
                    ctx_head(w - 1, 0, prev["ets"], prev)
                scores_cpair(w, 1, ets_pair)
                if w == 0:
                    # V projection emitted behind wave-0 scores: its DMAs
                    # arrive while K/Q/scores keep the PE busy and its
                    # matmuls fill the PE while wave-0 exp streams
                    vproj()
                if prev is not None:
                    norm_head(w - 1, 0, prev)
                    ctx_head(w - 1, 1, prev["ets"], prev)
                    norm_head(w - 1, 1, prev)
                    if (w - 1) % NPAIR == NPAIR - 1:
                        for mq in range(SQT // P):
                            op_queue.append(((w - 1) // NPAIR, prev["ctxt"], mq))
                pops = 0 if t == NSQT - 1 else 2
                for _ in range(pops):
                    if op_queue:
                        t_o, ctxt_o, mq = op_queue.pop(0)
                        outproj_group(t_o, ctxt_o, mq)
                if t + 1 < NSQT:
                    qproj_group(p, t + 1)

                prev = {"ets": ets_pair, **state}

            # drain: retire the last wave, interleaving the held-back tile-2
            # outproj groups between the normalization chain stages so the PE
            # stays busy while the last chains run
            w = NW - 1

            def pop_ops(k):
                for _ in range(k):
                    if op_queue:
                        t_o, ctxt_o, mq = op_queue.pop(0)
                        outproj_group(t_o, ctxt_o, mq, evac=nc.scalar)

            ctx_head(w, 0, prev["ets"], prev)
            pop_ops(1)
            ctx_head(w, 1, prev["ets"], prev)
            norm_head(w, 0, prev)
            pop_ops(2)
            norm_head(w, 1, prev)
            pop_ops(len(op_queue))
            for mq in range(SQT // P):
                outproj_group(NSQT - 1, prev["ctxt"], mq, evac=nc.scalar)

    nc.compile()
    return nc


def _get_nc(with_bias):
    key = ("bias" if with_bias else "fast")
    if key not in _CACHE:
        _CACHE[key] = _build_bass(with_bias)
    return _CACHE[key]


def _chunked(a):
    """[D, N] -> [P, D//P, N] with d = o*P + p  ->  [p][o][n], contiguous."""
    Dd, N = a.shape
    return np.ascontiguousarray(a.reshape(Dd // P, P, N).transpose(1, 0, 2))


def kernel(query, key, value, mask, uniform_set, Wq, bq, Wk, bk, Wv, bv, Wo, bo):
    import ml_dtypes
    from concourse import bass_utils

    bft = ml_dtypes.bfloat16
    f8t = ml_dtypes.float8_e4m3

    query = np.asarray(query, dtype=np.float32)
    key = np.asarray(key, dtype=np.float32)
    value = np.asarray(value, dtype=np.float32)
    mask = np.asarray(mask, dtype=np.float32)
    us = np.asarray(uniform_set).astype(bool)
    Wq = np.asarray(Wq, dtype=np.float32)
    Wk = np.asarray(Wk, dtype=np.float32)
    Wv = np.asarray(Wv, dtype=np.float32)
    Wo = np.asarray(Wo, dtype=np.float32)
    bq = np.asarray(bq, dtype=np.float32)
    bk = np.asarray(bk, dtype=np.float32)
    bv = np.asarray(bv, dtype=np.float32)
    bo = np.asarray(bo, dtype=np.float32)
    assert np.all(bq == 0.0), "kernel assumes bq == 0 (reference generates zeros)"

    scale = 1.0 / float(HD) ** 0.5
    wqt8_g = [
        _chunked(np.clip(Wq.T[:, g * DG : (g + 1) * DG] * (scale * SW8), -240, 240)).astype(f8t)
        for g in range(HG)
    ]
    wkt_g = [_chunked(Wk.T[:, g * DG : (g + 1) * DG]).astype(bft) for g in range(HG)]
    wvt_g = [_chunked(Wv.T[:, g * DG : (g + 1) * DG]).astype(bft) for g in range(HG)]
    wot_g = [
        np.ascontiguousarray(
            Wo.T[g * DG : (g + 1) * DG, :].reshape(MC, P, D).transpose(1, 0, 2)
        ).astype(bft)
        for g in range(HG)
    ]

    keeps = [us & (mask[b, 0, 0] >= 0) for b in range(B)]
    ns = [int(k.sum()) for k in keeps]
    with_bias = any(n < NK for n in ns)
    assert all(0 < n <= NK for n in ns), f"selected key counts {ns} unsupported"
    nc = _get_nc(with_bias)

    in_maps = []
    for b in range(B):
        idx = np.nonzero(keeps[b])[0]
        n = len(idx)
        kselt = np.zeros((D, NK), np.float32)
        kselt[:, :n] = key[b][idx].T
        vselt = np.zeros((D, NK), np.float32)
        vselt[:, :n] = value[b][idx].T
        kselt = _chunked(kselt).astype(bft)
        vselt = _chunked(vselt).astype(bft)
        # xt8: [NSQT, P, KC, SQT] fp8 (x * SX8)
        xt = _chunked(np.clip(query[b].T * SX8, -240, 240))  # [P, KC, S]
        xt8 = np.ascontiguousarray(
            xt.reshape(P, KC, NSQT, SQT).transpose(2, 0, 1, 3)
        ).astype(f8t)
        for g in range(HG):
            m = {
                "xt8": xt8,
                "kselt": kselt,
                "vselt": vselt,
                "wqt8": wqt8_g[g],
                "wkt": wkt_g[g],
                "wvt": wvt_g[g],
                "wot": wot_g[g],
            }
            if with_bias:
                kbias = np.full((NK,), -1e30, np.float32)
                kbias[:n] = 0.0
                m["kbias"] = np.ascontiguousarray(kbias.reshape(SC, P).T)
            in_maps.append(m)

    res = bass_utils.run_bass_kernel_spmd(nc, in_maps, core_ids=list(range(B * HG)))
    outs = [m["out"] for m in res.results]

    corr = (bo + Wo @ bv).astype(np.float32)
    out = np.empty((B, S, D), np.float32)
    for b in range(B):
        out[b] = (
            outs[HG * b].astype(np.float32)
            + outs[HG * b + 1].astype(np.float32)
            + corr
        )
    return out


# revision 47
# speedup vs baseline: 1.1871x; 1.0001x over previous
"""Sparse multi-head self-attention on 8 trn2 NeuronCores.

Problem: B=4, S=2048, D=768, H=12 heads of 64; only the 512 keys selected by
`uniform_set` (and not padding-masked) participate in attention.

Sharding: core = 2*b + hg  (b = batch 0..3, hg = head-group 0..1, 6 heads each,
Megatron-style column-sharded Wq/Wk/Wv + row-sharded Wo).  Each core computes a
partial output [S, D] for its batch from its 6 heads; host sums the two
head-group partials per batch.

Device algorithm (per core), all layouts transposed so no on-chip transposes:
  Qt[dout, s]  = Wq8^T . X8          fp8 DoubleRow (2 contraction chunks/pass,
                                     3 passes instead of 6 bf16 passes; host
                                     pre-scales both operands by powers of 2,
                                     undone in the PSUM->SBUF copy scale)
  Kt[dout, k]  = WkT . KselT         bf16, chunk-outer so the PE starts on the
                                     first arriving DMA chunk
  V  [k, dout] = VselT^T . WvT       bf16; V block layout per head is
                                     [den | pad63 | v64]: col 0 is the ones
                                     column (softmax denominator -> PSUM row 0,
                                     directly readable by the DVE reciprocal,
                                     which only supports PSUM base partition 0),
                                     v occupies cols 64..127 (64-aligned reads)
  scoresT[k, s] per head (64-contraction, PE row-tiled pairs); expT on ACT
  ctx'T[128, s] = vblk^T . expT   (row 0 = denominator d)
  rp = 1/d (DVE approx recip, reads PSUM row 0), partition-broadcast (gpsimd),
  ctxT = ctx'T[64:128] * rbb  (DVE, bf16 out)
  out partial[s_chunk, dout] = ctxT^T . WoT

Host pre-permutes every operand to the device SBUF layout so all input DMAs
are fully contiguous, and splits K-path DMAs into chunk pairs so the first
matmul can start ~1us after launch.

Biases: bq assumed 0 (reference generates zeros).  bk affects scores only via
per-query constants (softmax invariant).  bv and bo are applied exactly on the
host: out += bo + Wo @ bv (softmax weights sum to 1).
"""

import numpy as np

B, S, D, H, HD = 4, 2048, 768, 12, 64
HG = 2            # head groups (tensor parallel)
HPG = H // HG     # 6 heads per group
DG = HPG * HD     # 384 projection dims per group
NK = 512          # padded count of selected keys
P = 128
KC = D // P       # 6 contraction chunks over model dim
MC = DG // P      # 3 chunks of per-group projection dim
SC = NK // P      # 4 selected-key chunks
SQT = 512         # query-tile (moving free dim)
NSQT = S // SQT   # 4
NPAIR = MC        # 3 head-pairs per tile (pair p = heads 2p, 2p+1 in chunk p)

# fp8 Q-projection scales (powers of 2; product undone at PSUM evacuation)
SX8 = 8.0         # xt8 = x * SX8          (x std 1 -> 8)
SW8 = 4096.0      # wqt8 = Wq.T*scale*SW8  (std 0.0025 -> 10)
QEVAC = 1.0 / (SX8 * SW8)

_CACHE = {}


def _build_bass(with_bias):
    import concourse.mybir as mybir
    import concourse.tile as tile
    from concourse import bacc

    f32 = mybir.dt.float32
    bf16 = mybir.dt.bfloat16
    fp8 = mybir.dt.float8e4
    EXP = mybir.ActivationFunctionType.Exp
    COPY = mybir.ActivationFunctionType.Copy
    DRMODE = mybir.MatmulPerfMode.DoubleRow

    nc = bacc.Bacc("TRN2", name="sparse_mha")

    xt8_d = nc.dram_tensor("xt8", [NSQT, P, KC, SQT], fp8, kind="ExternalInput")
    wqt8_d = nc.dram_tensor("wqt8", [P, KC, DG], fp8, kind="ExternalInput")
    wkt_d = nc.dram_tensor("wkt", [P, KC, DG], bf16, kind="ExternalInput")
    kselt_d = nc.dram_tensor("kselt", [P, KC, NK], bf16, kind="ExternalInput")
    wvt_d = nc.dram_tensor("wvt", [P, KC, DG], bf16, kind="ExternalInput")
    vselt_d = nc.dram_tensor("vselt", [P, KC, NK], bf16, kind="ExternalInput")
    wot_d = nc.dram_tensor("wot", [P, MC, D], bf16, kind="ExternalInput")
    kb_d = nc.dram_tensor("kbias", [P, SC], f32, kind="ExternalInput") if with_bias else None
    out_d = nc.dram_tensor("out", [S, D], bf16, kind="ExternalOutput")

    with tile.TileContext(nc) as tc:
        with (
            tc.tile_pool(name="persist", bufs=1) as persist,
            tc.tile_pool(name="inputs", bufs=1) as inputs,
            tc.tile_pool(name="work", bufs=1) as work,
            tc.tile_pool(name="psum", bufs=1, space="PSUM") as psum,
        ):
            # ---- input DMAs: K path first (chunk pairs across two queues) so
            # the first matmuls start early; Q path on gpsimd queue in
            # parallel; V/Wo behind them ----
            if with_bias:
                kbias = persist.tile([P, SC], f32, tag="kbias")
                nc.gpsimd.dma_start(kbias, kb_d[:, :])

            # Critical path to wave-0 scores: kselt (full, chunk-paced) +
            # wkt m=0 slab + xt8 tile-0 + wqt8.  The m=1,2 K/Q work and V
            # are consumed later, inside wave 0.
            wkt = inputs.tile([P, KC, DG], bf16, tag="wkt")
            kselt = inputs.tile([P, KC, NK], bf16, tag="kselt")
            wqt8 = inputs.tile([P, KC, DG], fp8, tag="wqt8")
            xt8 = inputs.tile([P, NSQT, KC, SQT], fp8, tag="xt8")
            nc.sync.dma_start(kselt[:, 0:1, :], kselt_d[:, 0:1, :])
            nc.scalar.dma_start(wkt[:, 0:1, :], wkt_d[:, 0:1, :])
            nc.gpsimd.dma_start(wkt[:, 2:4, :], wkt_d[:, 2:4, :])
            nc.sync.dma_start(kselt[:, 1:2, :], kselt_d[:, 1:2, :])
            nc.scalar.dma_start(wkt[:, 1:2, :], wkt_d[:, 1:2, :])
            nc.sync.dma_start(kselt[:, 2:4, :], kselt_d[:, 2:4, :])
            nc.gpsimd.dma_start(wkt[:, 4:6, :], wkt_d[:, 4:6, :])
            nc.gpsimd.dma_start(kselt[:, 4:6, :], kselt_d[:, 4:6, :])

            nc.scalar.dma_start(xt8[:, 0, 3:6, :], xt8_d[0, :, 3:6, :])
            nc.gpsimd.dma_start(wqt8, wqt8_d[:, :, :])
            nc.sync.dma_start(xt8[:, 0, 0:3, :], xt8_d[0, :, 0:3, :])

            # V path, later query tiles
            wvt = inputs.tile([P, KC, DG], bf16, tag="wvt")
            vselt = inputs.tile([P, KC, NK], bf16, tag="vselt")
            nc.sync.dma_start(vselt[:, 0:3, :], vselt_d[:, 0:3, :])
            nc.gpsimd.dma_start(vselt[:, 3:6, :], vselt_d[:, 3:6, :])
            nc.scalar.dma_start(wvt, wvt_d[:, :, :])

            wot = persist.tile([P, MC, D], bf16, tag="wot")
            nc.gpsimd.dma_start(xt8[:, 1, :, :], xt8_d[1, :, :, :])
            nc.scalar.dma_start(wot, wot_d[:, :, :])
            nc.sync.dma_start(xt8[:, 2, :, :], xt8_d[2, :, :, :])
            nc.scalar.dma_start(xt8[:, 3, :, :], xt8_d[3, :, :, :])

            # V blocks: [P(sk), SC, HPG, 128]; col 0 = ones (denominator row),
            # cols 1..63 zero pad, cols 64..128 = v
            vb = persist.tile([P, SC, HPG, P], bf16, tag="vb")
            nc.vector.memset(vb[:, :, :, 0:HD], 0.0)
            nc.vector.memset(vb[:, :, :, 0:1], 1.0)

            ktp = persist.tile([P, MC, NK], bf16, tag="ktp")
            qt = persist.tile([P, MC, S], bf16, tag="qt")

            # ---- warmup projections ----
            # K: chunk-outer over 3 open PSUM groups, paced by the kselt
            # chunk DMAs
            psK = [psum.tile([P, SQT], f32, tag="ctx", bufs=3, name=f"kp{m}") for m in range(MC)]
            for i in range(KC):
                for m in range(MC):
                    nc.tensor.matmul(
                        psK[m][:, :NK],
                        lhsT=wkt[:, i, m * P : (m + 1) * P],
                        rhs=kselt[:, i, :],
                        start=(i == 0),
                        stop=(i == KC - 1),
                    )
            for m in range(MC):
                nc.vector.tensor_copy(ktp[:, m, :], psK[m][:, :NK])

            def qproj_group(m, t):
                sq = slice(t * SQT, (t + 1) * SQT)
                ps = psum.tile([P, SQT], f32, tag="pj", bufs=1, name=f"qp{t}_{m}")
                for j in range(MC):
                    nc.tensor.matmul(
                        ps[:, :SQT],
                        lhsT=wqt8[:, 2 * j : 2 * j + 2, m * P : (m + 1) * P],
                        rhs=xt8[:, t, 2 * j : 2 * j + 2, :],
                        start=(j == 0),
                        stop=(j == MC - 1),
                        perf_mode=DRMODE,
                    )
                nc.scalar.activation(qt[:, m, sq], ps[:, :SQT], COPY, 0.0, QEVAC)

            for m in range(MC):
                qproj_group(m, 0)

            def vproj():
                # V: chunk-outer, 4 open groups in the 2 sc-ring tiles
                psV = [psum.tile([P, 2 * SQT], f32, tag="sc", bufs=2, name=f"vp{cc}") for cc in range(2)]
                for i in range(KC):
                    for c in range(SC):
                        nc.tensor.matmul(
                            psV[c // 2][:, (c % 2) * SQT : (c % 2) * SQT + DG],
                            lhsT=vselt[:, i, c * P : (c + 1) * P],
                            rhs=wvt[:, i, :],
                            start=(i == 0),
                            stop=(i == KC - 1),
                        )
                for c in range(SC):
                    nc.vector.tensor_copy(
                        vb[:, c, :, HD:P],
                        psV[c // 2][:, (c % 2) * SQT : (c % 2) * SQT + DG].rearrange(
                            "p (h d) -> p h d", h=HPG
                        ),
                    )

            # ---- out-projection (per query-chunk of 128); output DMAs
            # alternate between the sync and gpsimd rings so the final
            # cluster drains in parallel ----
            op_count = [0]

            def outproj_group(t_o, ctxt_o, mq, evac=None):
                sq0 = t_o * SQT + mq * P
                op = psum.tile([P, 2 * SQT], f32, tag="sc", bufs=2, name=f"op{t_o}_{mq}")
                for n in range(2):
                    for j2 in range(MC):
                        nc.tensor.matmul(
                            op[:, n * SQT : n * SQT + 384],
                            lhsT=ctxt_o[:, j2, mq * P : (mq + 1) * P],
                            rhs=wot[:, j2, n * 384 : (n + 1) * 384],
                            start=(j2 == 0),
                            stop=(j2 == MC - 1),
                        )
                ot = work.tile([P, D], bf16, tag="ot", bufs=6, name=f"ot{t_o}_{mq}")
                src = op.rearrange("p (n x) -> p n x", n=2)[:, :, 0:384]
                dst = ot.rearrange("p (n x) -> p n x", n=2)
                if evac is None:
                    nc.vector.tensor_copy(dst, src)
                else:
                    evac.copy(dst, src)
                op_count[0] += 1
                if evac is not None:
                    # drain phase: split across both rings to halve the tail
                    nc.sync.dma_start(out_d[sq0 : sq0 + 64, :], ot[0:64, :])
                    nc.gpsimd.dma_start(out_d[sq0 + 64 : sq0 + P, :], ot[64:P, :])
                else:
                    eng = nc.sync if op_count[0] % 2 == 0 else nc.gpsimd
                    eng.dma_start(out_d[sq0 : sq0 + P, :], ot)

            # ---- attention waves ----
            NW = NSQT * NPAIR

            def scores_cpair(w, j, ets_pair):
                t, p = divmod(w, NPAIR)
                sq = slice(t * SQT, (t + 1) * SQT)
                sct = [
                    psum.tile([P, 2 * SQT], f32, tag="sc", bufs=2, name=f"sc{w}_{j}_{hi}")
                    for hi in range(2)
                ]
                for cc in range(2):
                    c = 2 * j + cc
                    for hi in range(2):
                        lo = 64 * hi
                        nc.tensor.matmul(
                            sct[hi][:, cc * SQT : (cc + 1) * SQT],
                            lhsT=ktp[lo : lo + 64, p, c * P : (c + 1) * P],
                            rhs=qt[lo : lo + 64, p, sq],
                            start=True,
                            stop=True,
                        )
                for hi in range(2):
                    if with_bias:
                        for cc in range(2):
                            c = 2 * j + cc
                            nc.scalar.activation(
                                out=ets_pair[hi][:, c, :],
                                in_=sct[hi][:, cc * SQT : (cc + 1) * SQT],
                                func=EXP,
                                bias=kbias[:, c : c + 1],
                                scale=1.0,
                            )
                    else:
                        nc.scalar.activation(
                            out=ets_pair[hi][:, 2 * j : 2 * j + 2, :],
                            in_=sct[hi].rearrange("p (c q) -> p c q", c=2),
                            func=EXP,
                        )

            def ctx_part(w, hi, ets_pair, state, cs):
                t, p = divmod(w, NPAIR)
                h = 2 * p + hi
                if cs[0] == 0:
                    state["cp"][hi] = psum.tile(
                        [P, SQT], f32, tag="ctx", bufs=3, name=f"cp{w}_{hi}"
                    )
                cp = state["cp"][hi]
                for c in cs:
                    nc.tensor.matmul(
                        cp[:, :],
                        lhsT=vb[:, c, h, :],
                        rhs=ets_pair[hi][:, c, :],
                        start=(c == 0),
                        stop=(c == SC - 1),
                    )
                if cs[-1] != SC - 1:
                    return
                # 1/d directly from PSUM row 0 (DVE), partition-broadcast
                sl = slice(hi * SQT, (hi + 1) * SQT)
                rp, rbb = state["rp"], state["rbb"]
                nc.vector.reciprocal_approx_fast(rp[0:1, sl], cp[0:1, :])
                nc.gpsimd.partition_broadcast(rbb[0:HD, sl], rp[0:1, sl])

            def ctx_head(w, hi, ets_pair, state):
                ctx_part(w, hi, ets_pair, state, [0, 1])
                ctx_part(w, hi, ets_pair, state, [2, 3])

            def norm_head(w, hi, state):
                t, p = divmod(w, NPAIR)
                lo = 64 * hi
                nc.vector.tensor_tensor(
                    state["ctxt"][lo : lo + 64, p, :],
                    state["cp"][hi][64:P, :],
                    state["rbb"][0:HD, hi * SQT : (hi + 1) * SQT],
                    op=mybir.AluOpType.mult,
                )

            # steady-state emission: per window issue scores(w), retire w-1,
            # weave outproj of tile t-1 and one qproj group for tile t+1
            op_queue = []
            prev = None
            for w in range(NW):
                t, p = divmod(w, NPAIR)
                ets_pair = [
                    work.tile([P, SC, SQT], bf16, tag="ets", bufs=6, name=f"et{w}_{hi}")
                    for hi in range(2)
                ]
                state = {
                    "rp": work.tile([1, 2 * SQT], f32, tag="rp", bufs=3, name=f"rp{w}"),
                    "rbb": work.tile([HD, 2 * SQT], f32, tag="rbb", bufs=3, name=f"rbb{w}"),
                    "cp": [None, None],
                    "ctxt": (
                        prev["ctxt"]
                        if p != 0
                        else work.tile([P, MC, SQT], bf16, tag="ctxt", bufs=4, name=f"ctxt{t}")
                    ),
                }

                scores_cpair(w, 0, ets_pair)
                if prev is not None:
                    ctx_head(w - 1, 0, prev["ets"], prev)
                scores_cpair(w, 1, ets_pair)
                if w == 0:
                    # V projection emitted behind wave-0 scores: its DMAs
                    # arrive while K/Q/scores keep the PE busy and its
                    # matmuls fill the PE while wave-0 exp streams
                    vproj()
                if prev is not None:
                    norm_head(w - 1, 0, prev)
                    ctx_head(w - 1, 1, prev["ets"], prev)
                    norm_head(w - 1, 1, prev)
                    if (w - 1) % NPAIR == NPAIR - 1:
                        for mq in range(SQT // P):
                            op_queue.append(((w - 1) // NPAIR, prev["ctxt"], mq))
                pops = 0 if t == NSQT - 1 else 2
                for _ in range(pops):
                    if op_queue:
                        t_o, ctxt_o, mq = op_queue.pop(0)
                        outproj_group(t_o, ctxt_o, mq)
                if t + 1 < NSQT:
                    qproj_group(p, t + 1)

                prev = {"ets": ets_pair, **state}

            # drain: retire the last wave, interleaving the held-back tile-2
            # outproj groups between the normalization chain stages so the PE
            # stays busy while the last chains run
            w = NW - 1

            def pop_ops(k):
                for _ in range(k):
                    if op_queue:
                        t_o, ctxt_o, mq = op_queue.pop(0)
                        outproj_group(t_o, ctxt_o, mq, evac=nc.scalar)

            ctx_head(w, 0, prev["ets"], prev)
            pop_ops(1)
            ctx_head(w, 1, prev["ets"], prev)
            norm_head(w, 0, prev)
            pop_ops(2)
            norm_head(w, 1, prev)
            pop_ops(len(op_queue))
            for mq in range(SQT // P):
                outproj_group(NSQT - 1, prev["ctxt"], mq, evac=nc.scalar)

    nc.compile()
    return nc


def _get_nc(with_bias):
    key = ("bias" if with_bias else "fast")
    if key not in _CACHE:
        _CACHE[key] = _build_bass(with_bias)
    return _CACHE[key]


def _chunked(a):
    """[D, N] -> [P, D//P, N] with d = o*P + p  ->  [p][o][n], contiguous."""
    Dd, N = a.shape
    return np.ascontiguousarray(a.reshape(Dd // P, P, N).transpose(1, 0, 2))


def kernel(query, key, value, mask, uniform_set, Wq, bq, Wk, bk, Wv, bv, Wo, bo):
    import ml_dtypes
    from concourse import bass_utils

    bft = ml_dtypes.bfloat16
    f8t = ml_dtypes.float8_e4m3

    query = np.asarray(query, dtype=np.float32)
    key = np.asarray(key, dtype=np.float32)
    value = np.asarray(value, dtype=np.float32)
    mask = np.asarray(mask, dtype=np.float32)
    us = np.asarray(uniform_set).astype(bool)
    Wq = np.asarray(Wq, dtype=np.float32)
    Wk = np.asarray(Wk, dtype=np.float32)
    Wv = np.asarray(Wv, dtype=np.float32)
    Wo = np.asarray(Wo, dtype=np.float32)
    bq = np.asarray(bq, dtype=np.float32)
    bk = np.asarray(bk, dtype=np.float32)
    bv = np.asarray(bv, dtype=np.float32)
    bo = np.asarray(bo, dtype=np.float32)
    assert np.all(bq == 0.0), "kernel assumes bq == 0 (reference generates zeros)"

    scale = 1.0 / float(HD) ** 0.5
    wqt8_g = [
        _chunked(np.clip(Wq.T[:, g * DG : (g + 1) * DG] * (scale * SW8), -240, 240)).astype(f8t)
        for g in range(HG)
    ]
    wkt_g = [_chunked(Wk.T[:, g * DG : (g + 1) * DG]).astype(bft) for g in range(HG)]
    wvt_g = [_chunked(Wv.T[:, g * DG : (g + 1) * DG]).astype(bft) for g in range(HG)]
    wot_g = [
        np.ascontiguousarray(
            Wo.T[g * DG : (g + 1) * DG, :].reshape(MC, P, D).transpose(1, 0, 2)
        ).astype(bft)
        for g in range(HG)
    ]

    keeps = [us & (mask[b, 0, 0] >= 0) for b in range(B)]
    ns = [int(k.sum()) for k in keeps]
    with_bias = any(n < NK for n in ns)
    assert all(0 < n <= NK for n in ns), f"selected key counts {ns} unsupported"
    nc = _get_nc(with_bias)

    in_maps = []
    for b in range(B):
        idx = np.nonzero(keeps[b])[0]
        n = len(idx)
        kselt = np.zeros((D, NK), np.float32)
        kselt[:, :n] = key[b][idx].T
        vselt = np.zeros((D, NK), np.float32)
        vselt[:, :n] = value[b][idx].T
        kselt = _chunked(kselt).astype(bft)
        vselt = _chunked(vselt).astype(bft)
        # xt8: [NSQT, P, KC, SQT] fp8 (x * SX8)
        xt = _chunked(np.clip(query[b].T * SX8, -240, 240))  # [P, KC, S]
        xt8 = np.ascontiguousarray(
            xt.reshape(P, KC, NSQT, SQT).transpose(2, 0, 1, 3)
        ).astype(f8t)
        for g in range(HG):
            m = {
                "xt8": xt8,
                "kselt": kselt,
                "vselt": vselt,
                "wqt8": wqt8_g[g],
                "wkt": wkt_g[g],
                "wvt": wvt_g[g],
                "wot": wot_g[g],
            }
            if with_bias:
                kbias = np.full((NK,), -1e30, np.float32)
                kbias[:n] = 0.0
                m["kbias"] = np.ascontiguousarray(kbias.reshape(SC, P).T)
            in_maps.append(m)

    res = bass_utils.run_bass_kernel_spmd(nc, in_maps, core_ids=list(range(B * HG)))
    outs = [m["out"] for m in res.results]

    corr = (bo + Wo @ bv).astype(np.float32)
    out = np.empty((B, S, D), np.float32)
    for b in range(B):
        out[b] = (
            outs[HG * b].astype(np.float32)
            + outs[HG * b + 1].astype(np.float32)
            + corr
        )
    return out
